# revision 25
# baseline (speedup 1.0000x reference)
"""Trainium2 Bass kernel for nn_Conv2d (B=32, 256->256, 56x56, 3x3, pad=1) + bias.

Strategy
--------
Data-parallel over batch: 4 images per NeuronCore x 8 cores; weights/bias
replicated; no collectives.

Per core, the conv is computed as shifted matmuls on a host-padded input
(59 rows x 58 cols per image-channel): the tap (kh, kw) contribution to
output row-block [8n, 8n+8) is a matmul whose moving operand is the 2D
window x[:, 8n+kh : 8n+kh+8, kw : kw+56] (row stride 58) and whose PSUM
output is [8, 56] contiguous — so no junk columns are ever computed.
Operands are bf16 (rel err ~4e-3 vs the 2e-2 gate): bf16 self-loading
matmuls hide their LDWEIGHTS under the previous matmul's stream, unlike
f32r.  Each (img, cout-chunk) accumulates 18 (cin-chunk, tap) matmuls per
PSUM bank; banks are grouped 4+3 and ping-pong across the 8 PSUM banks so
evictions (bias-add on DVE, bf16 out) and stores overlap the next group's
matmuls.  Dummy matmuls on a memset tile warm the PE clock (p-state) while
the first DMAs land (~5us fixed DMA-engine startup latency).
"""

import numpy as np
import ml_dtypes

import concourse.bacc as bacc
import concourse.tile as tile
import concourse.mybir as mybir
from concourse.bass_utils import run_bass_kernel_spmd

F32 = mybir.dt.float32
BF16 = mybir.dt.bfloat16
BF = ml_dtypes.bfloat16

B, CIN, COUT, H, W, K = 32, 256, 256, 56, 56, 3
NCORES = 8
BPC = B // NCORES          # images per core
WP = W + 2                 # padded row width (58)
HP = H + 3                 # padded rows (59): 1 top, 2 bottom (tail tap reads)
NT = 7                     # output row-blocks per (img, cout-chunk)
RB = H // NT               # 8 output rows per block
NFREE = RB * W             # 448 output positions per matmul
NW = K * K * 128           # weight free length per (ci, cc): 9 taps x 128 couts

_CACHE = {}


def _build():
    if "nc" in _CACHE:
        return _CACHE["nc"]
    nc = bacc.Bacc("TRN2", target_bir_lowering=False, debug=False,
                   num_swdge_queues=1)
    x_d = nc.dram_tensor("x", [BPC, CIN, HP, WP], BF16,
                         kind="ExternalInput").ap()
    w_d = nc.dram_tensor("w", [2, 128, 2, NW], BF16, kind="ExternalInput").ap()
    b_d = nc.dram_tensor("b", [COUT], F32, kind="ExternalInput").ap()
    o_d = nc.dram_tensor("o", [BPC, COUT, H, W], BF16,
                         kind="ExternalOutput").ap()

    with tile.TileContext(nc) as tc:
        with (
            tc.tile_pool(name="wp", bufs=1) as wp,
            tc.tile_pool(name="xp", bufs=4) as xp,
            tc.tile_pool(name="op", bufs=2) as op,
            tc.tile_pool(name="pp", bufs=8, space="PSUM") as pp,
        ):
            # DMA trigger instructions cost ~0.65us EACH on the issuing
            # engine, so issue in parallel from both HWDGE engines:
            # sync carries ci=0 traffic, scalar carries ci=1.
            eng = [nc.sync, nc.scalar]

            bias_t = wp.tile([128, 2], F32)
            # weights [cin-in-chunk, ci, cc, tap*128+cout]: one contiguous
            # 295KB DMA per (ci, cc) chunk, split 3+6 taps for cc=0.
            w_t = wp.tile([128, 2, 2, NW], BF16)

            def w_dma(e, ci, cc, lo=0, hi=K * K):
                e.dma_start(out=w_t[:, ci, cc, lo * 128:hi * 128],
                            in_=w_d[ci, :, cc, lo * 128:hi * 128])

            def x_dma(e, xs, img, ci, lo, hi):
                e.dma_start(
                    out=xs[ci][:, lo:hi, :],
                    in_=x_d[img, ci * 128:(ci + 1) * 128, lo:hi, :],
                )

            # steady images: 4 coarse row-slices (they prefetch a whole
            # image ahead anyway)
            xsl = [0, 15, 30, 45, HP]

            def load_img0():
                # Hand-scheduled by need-by time: mi walks ci0 taps 0-8
                # (~0.75us each) then ci1, so ci=1 data has ~7us of slack —
                # the scalar engine fronts ci0's mid rows while sync lands
                # the critical rows [0:8)+w taps that gate the first matmul.
                xs = [xp.tile([128, HP, WP], BF16, tag="x", name=f"x_0_{ci}")
                      for ci in range(2)]
                s0, s1 = eng

                def X(e, ci, a, b):
                    x_dma(e, xs, 0, ci, a, b)

                # the first matmul wave (4 row-blocks) touches ci0 rows
                # 0-33 within ~1us, so scalar fronts rows 18-35 while sync
                # lands rows 0-18 + the gating weight taps
                X(s0, 0, 0, 8); w_dma(s0, 0, 0, 0, 1)
                X(s0, 0, 18, 27); w_dma(s0, 0, 0, 1, 3)
                w_dma(s0, 0, 0, 3, K * K)
                X(s0, 0, 35, 44); X(s0, 0, 44, HP)
                s0.dma_start(out=bias_t[:, 0:1], in_=b_d[0:128])
                w_dma(s0, 0, 1)

                X(s1, 0, 8, 18); X(s1, 0, 27, 35)
                X(s1, 1, 0, 8); w_dma(s1, 1, 0, 0, 3)
                X(s1, 1, 8, 18); X(s1, 1, 18, 27)
                w_dma(s1, 1, 0, 3, K * K)
                X(s1, 1, 27, 44); X(s1, 1, 44, HP)
                s1.dma_start(out=bias_t[:, 1:2], in_=b_d[128:256])
                w_dma(s1, 1, 1)
                return xs

            def load_img(img, first=False):
                if first:
                    return load_img0()
                xs = [xp.tile([128, HP, WP], BF16, tag="x",
                              name=f"x_{img}_{ci}") for ci in range(2)]
                for ci in range(2):
                    e = eng[ci]
                    for s in range(len(xsl) - 1):
                        x_dma(e, xs, img, ci, xsl[s], xsl[s + 1])
                return xs

            def do_pass(xs, cc, o_t, img, nts, fine=False):
                """One PSUM accumulation wave over row-blocks `nts`: 18
                (ci, tap) x len(nts) matmuls, weight-outermost so the bf16
                self-load LDWEIGHTS amortizes over len(nts) streams."""
                pss = [pp.tile([128, RB, W], F32, tag="ps",
                               name=f"ps_{img}_{cc}_{nt}") for nt in nts]
                for mi, (ci, t) in enumerate(
                    [(ci, t) for ci in range(2) for t in range(K * K)]
                ):
                    kh, kw = divmod(t, K)
                    wsl = w_t[:, ci, cc, t * 128:(t + 1) * 128]
                    for ps, nt in zip(pss, nts):
                        r0 = nt * RB + kh
                        nc.tensor.matmul(
                            ps, wsl, xs[ci][:, r0:r0 + RB, kw:kw + W],
                            start=(mi == 0), stop=(mi == 17),
                        )
                # bias-add + PSUM eviction on the otherwise-idle DVE,
                # bf16 out halves store traffic
                for j, (ps, nt) in enumerate(zip(pss, nts)):
                    if fine and len(nts) == 1:
                        # final bank: half-row-block evicts + stores on both
                        # engines to shorten the drain tail
                        hb = RB // 2
                        for h in range(2):
                            r = nt * RB + h * hb
                            if h == 0:
                                nc.vector.tensor_scalar_add(
                                    o_t[:, r:r + hb, :],
                                    ps[:, h * hb:(h + 1) * hb, :],
                                    bias_t[:, cc:cc + 1],
                                )
                            else:
                                # second half on ScalarE so both evicts run
                                # concurrently in the drain tail
                                nc.scalar.activation(
                                    o_t[:, r:r + hb, :],
                                    ps[:, h * hb:(h + 1) * hb, :],
                                    mybir.ActivationFunctionType.Identity,
                                    bias=bias_t[:, cc:cc + 1],
                                )
                            eng[h].dma_start(
                                out=o_d[img, cc * 128:(cc + 1) * 128,
                                        r:r + hb, :],
                                in_=o_t[:, r:r + hb, :],
                            )
                        continue
                    nc.vector.tensor_scalar_add(
                        o_t[:, nt * RB:(nt + 1) * RB, :],
                        ps,
                        bias_t[:, cc:cc + 1],
                    )
                    if fine:
                        eng[j % 2].dma_start(
                            out=o_d[img, cc * 128:(cc + 1) * 128,
                                    nt * RB:(nt + 1) * RB, :],
                            in_=o_t[:, nt * RB:(nt + 1) * RB, :],
                        )
                if not fine:
                    # region A -> sync, region B -> scalar: keeps either
                    # HWDGE queue from accumulating all store transfers
                    lo, hi = nts[0] * RB, (nts[-1] + 1) * RB
                    eng[(nts[0] // 4) % 2].dma_start(
                        out=o_d[img, cc * 128:(cc + 1) * 128, lo:hi, :],
                        in_=o_t[:, lo:hi, :],
                    )

            def do_last_block(xs, cc, o_t, img, nt):
                """Final row-block as two sequential half-block accumulation
                chains, so the first half's evict+store overlap the second
                half's matmuls and only a 4-row store sits in the drain."""
                ps = pp.tile([128, RB, W], F32, tag="ps", name=f"ps_last{cc}")
                hb = RB // 2
                for h in (1, 0):
                    for mi, (ci, t) in enumerate(
                        [(ci, t) for ci in range(2) for t in range(K * K)]
                    ):
                        kh, kw = divmod(t, K)
                        wsl = w_t[:, ci, cc, t * 128:(t + 1) * 128]
                        r0 = nt * RB + h * hb + kh
                        nc.tensor.matmul(
                            ps[:, h * hb:(h + 1) * hb, :], wsl,
                            xs[ci][:, r0:r0 + hb, kw:kw + W],
                            start=(mi == 0), stop=(mi == 17),
                        )
                    r = nt * RB + h * hb
                    if h == 1:
                        nc.vector.tensor_scalar_add(
                            o_t[:, r:r + hb, :],
                            ps[:, h * hb:(h + 1) * hb, :],
                            bias_t[:, cc:cc + 1],
                        )
                    else:
                        nc.scalar.activation(
                            o_t[:, r:r + hb, :],
                            ps[:, h * hb:(h + 1) * hb, :],
                            mybir.ActivationFunctionType.Identity,
                            bias=bias_t[:, cc:cc + 1],
                        )
                    eng[h].dma_start(
                        out=o_d[img, cc * 128:(cc + 1) * 128, r:r + hb, :],
                        in_=o_t[:, r:r + hb, :],
                    )

            # warm the PE clock (p-state ramps over ~9us of activity) with
            # dummy matmuls on a memset tile while the first DMAs land
            # (~5.5us trigger-to-completion on a cold DMA engine)
            wu = wp.tile([128, NFREE], BF16)
            nc.gpsimd.memset(wu, 0)
            ps_warm = pp.tile([128, NFREE], F32, tag="ps", name="ps_warm")
            for _ in range(11):
                nc.tensor.matmul(ps_warm, wu[:, 0:128], wu,
                                 start=True, stop=True)

            for img in range(BPC):
                xs = load_img(img, first=(img == 0))
                for cc in range(2):
                    o_t = op.tile([128, H, W], BF16, tag="o",
                                  name=f"o_{img}_{cc}")
                    last = img == BPC - 1 and cc == 1
                    if last:
                        # taper the final passes so the drain tail is short
                        do_pass(xs, cc, o_t, img, [0, 1, 2, 3])
                        do_pass(xs, cc, o_t, img, [4, 5], fine=True)
                        do_last_block(xs, cc, o_t, img, 6)
                    else:
                        do_pass(xs, cc, o_t, img, [0, 1, 2, 3])
                        do_pass(xs, cc, o_t, img, [4, 5, 6])
    nc.compile()
    _CACHE["nc"] = nc
    return nc


def make_in_maps(inp, kernel, bias):
    xpad = np.zeros((B, CIN, HP, WP), np.float32)
    xpad[:, :, 1:1 + H, 1:1 + W] = inp
    xdev = xpad.astype(BF)
    # [cout, cin, kh, kw] -> [ci, cin_in, cc, tap*128+cout_in]
    kk = np.asarray(kernel, np.float32).reshape(2, 128, 2, 128, K, K)
    w_dev = np.ascontiguousarray(
        kk.transpose(2, 3, 0, 4, 5, 1).reshape(2, 128, 2, NW)
    ).astype(BF)
    b_dev = np.ascontiguousarray(np.asarray(bias, np.float32))
    return [
        {"x": np.ascontiguousarray(xdev[c * BPC:(c + 1) * BPC]),
         "w": w_dev, "b": b_dev}
        for c in range(NCORES)
    ]


def assemble(results):
    o = np.concatenate([np.asarray(results[c]["o"]) for c in range(NCORES)],
                       axis=0)
    return np.ascontiguousarray(o.astype(np.float32))


def kernel(inp, kernel, bias):
    nc = _build()
    in_maps = make_in_maps(inp, kernel, bias)
    r = run_bass_kernel_spmd(nc, in_maps, core_ids=list(range(NCORES)))
    return assemble(r.results)
